# revision 1
# baseline (speedup 1.0000x reference)
"""Trainium2 Bass kernel for DCKModule (involution / dynamic per-pixel conv).

Math (per image, 1 image per core, 8 cores data-parallel over batch):
  x  = relu(W1p @ guide + bias)                  # (64, 9216)
  df = W2 @ x                                    # (784 = 16 g * 49 tap, 9216)
  out[c,r,j] = sum_k df[g(c),k,r,j] * fpad[c, r+di(k), j+dj(k)] + feature

Mapping (all fp16 data, fp32 PSUM accumulation):
- ROW-partition layout: image rows on SBUF partitions. A tap's row shift di
  is a partition offset of the padded-feature operand (DVE allows partition
  offsets; PE does not), the col shift dj is a free-dim offset.
- df is computed once per pixel (NOT broadcast x16 over group channels):
  per column j, PE computes x_col^T @ W2a^T -> df_j PSUM [96 r, 784], and
  ACT transpose-converts it into fp16 j-slab tiles laid out (k, g, j12).
  The x16 group-channel broadcast is free: the DVE mult reads df through a
  stride-0 AP dim.
- DVE does only the per-tap MULT, in fp16 with all-SBUF operands -> 2x DVE
  perf mode (0.52 ns/elem-row vs 1.04 fp32).
- The tap ACCUMULATION runs on the otherwise-idle PE as identity matmuls
  accumulating in PSUM fp32 (start at tap 0, stop at tap 48).
- The residual (+feature) is folded into df: x gets a constant-1 row 64 and
  W2a^T gets a row that adds 1.0 to every group's center tap (k=24).
- Output is DMA'd straight from PSUM in production order [96 r, js, ch,
  c128, j12]; the host unshuffles to (256, 96, 96). All transposes/pads of
  inputs likewise happen host-side for free.

Predicted engine busy per core (TimelineSim model): DVE ~675us (bottleneck),
PE ~570us, ACT ~100us, DMA ~55us.
"""

import numpy as np

import concourse.bass as bass
import concourse.mybir as mybir
import concourse.tile as tile
from concourse import bacc, bass_utils

B, C, H, W = 8, 256, 96, 96
K7, PAD, G, GC, R = 7, 3, 16, 16, 64
HP = H + 2 * PAD          # 102
PIX = H * W               # 9216
BN_EPS = 1e-5
JS = 12                   # j-slab width (output cols per slab)
NJS = W // JS             # 8
CH = 2                    # channel halves (128 each)
CHW = C // CH             # 128
GH = G // CH              # 8 groups per half
CENTER = PAD * K7 + PAD   # 24

F32 = mybir.dt.float32
F16 = mybir.dt.float16
TRACE = False

_CACHE = {}


FW = JS + K7 - 1          # 18: slab cols incl dj halo
FCH = K7 * CHW * FW       # 16128: one (js, ch) fpad chunk per partition


def _build_nc():
    nc = bacc.Bacc(None, target_bir_lowering=False)
    # 7 row-shifted copies of padded feature, chunked (js, ch, di, c, j):
    # DVE/PE operands must start at partition 0/32/64/96, so the tap row
    # shift di is materialized host-side instead of via partition offsets
    fpad_d = nc.dram_tensor("fpd7", [H, NJS * CH * FCH], F16,
                            kind="ExternalInput")
    gm_d = nc.dram_tensor("gm", [C, PIX], F16, kind="ExternalInput")
    w1_d = nc.dram_tensor("w1t", [C, R], F16, kind="ExternalInput")
    bias_d = nc.dram_tensor("bias", [R, 1], F32, kind="ExternalInput")
    w2_d = nc.dram_tensor("w2ta", [R + 1, G * K7 * K7], F16,
                          kind="ExternalInput")
    i96_d = nc.dram_tensor("i96", [H, H], F16, kind="ExternalInput")
    ones_d = nc.dram_tensor("ones", [1, PIX], F16, kind="ExternalInput")
    out_d = nc.dram_tensor("out", [H, C * W], F32, kind="ExternalOutput")

    NO = G * K7 * K7      # 784
    with tile.TileContext(nc) as tc:
        with tc.tile_pool(name="persist", bufs=1) as persist, \
             tc.tile_pool(name="dfpool", bufs=2) as dfpool, \
             tc.tile_pool(name="prodpool", bufs=8) as prodpool, \
             tc.tile_pool(name="pprodpool", bufs=3) as pprodpool, \
             tc.tile_pool(name="outpool", bufs=2) as outpool, \
             tc.tile_pool(name="fpool", bufs=2) as fpool, \
             tc.tile_pool(name="ps", bufs=1, space="PSUM") as ps:

            gm_sb = [persist.tile([128, PIX], F16, tag=f"gm{ct}",
                                  name=f"gm{ct}") for ct in range(2)]
            w1_sb = persist.tile([128, 2 * R], F16, tag="w1", name="w1sb")
            bias_sb = persist.tile([R, 1], F32, tag="bias", name="biassb")
            w2_sb = persist.tile([R + 1, NO], F16, tag="w2", name="w2sb")
            i96_sb = persist.tile([H, H], F16, tag="i96", name="i96sb")
            x_sb = persist.tile([R + 1, PIX], F16, tag="x", name="xsb")

            # DMA order tuned against the shared DMA device: small weights
            # first, then the gm chunks that unblock x/df(0), then fpad rows
            # 0..95 (first needed by the first tap ~22us in), then the rest
            nc.sync.dma_start(out=w1_sb[:, 0:R], in_=w1_d[0:128, :])
            nc.sync.dma_start(out=w1_sb[:, R:2 * R], in_=w1_d[128:256, :])
            nc.sync.dma_start(out=bias_sb[:], in_=bias_d[:])
            nc.sync.dma_start(out=w2_sb[:], in_=w2_d[:])
            # constant-1 row of x folds the +feature residual into df; a
            # DVE memset here would gate the whole df chain by ~10us
            nc.sync.dma_start(out=x_sb[R:R + 1, :], in_=ones_d[:])

            def fetch_chunk(js, ch):
                """DMA one (js, ch) fpad chunk: [96, (di 7, c 128, j 18)]."""
                fch = fpool.tile([H, FCH], F16, tag="fch", name="fch")
                base = (js * CH + ch) * FCH
                nc.sync.dma_start(out=fch[:],
                                  in_=fpad_d[:, base:base + FCH])
                return fch[:].rearrange("p (di c j) -> p di c j",
                                        di=K7, c=CHW)

            fcur = None
            bounds = [0, 1536, 3072, 6144, PIX]
            for gc_ in range(4):
                lo, hi = bounds[gc_], bounds[gc_ + 1]
                for ct in range(2):
                    nc.sync.dma_start(out=gm_sb[ct][:, lo:hi],
                                      in_=gm_d[ct * 128:(ct + 1) * 128,
                                               lo:hi])
                if gc_ == 0:
                    nc.sync.dma_start(out=i96_sb[:], in_=i96_d[:])
                elif gc_ == 1:
                    fcur = fetch_chunk(0, 0)

            # ---- phase 1: x = relu(W1p @ guide + bias), fp16 ----
            # guide/x use j-major pixel order (pix = j*96 + r) so df columns
            # are contiguous x slices and df(0) can start after 3 x-chunks
            XCH = 512

            def x_chunk(s, borrow=False):
                if borrow:
                    px = ps.tile([H, 3 * 512], F32, tag="acc", name="px")
                else:
                    px = ps.tile([H, 1024], F32, tag="df", name="dfp")
                for ct in range(2):
                    nc.tensor.matmul(
                        px[:R, :XCH], w1_sb[:, ct * R:(ct + 1) * R],
                        gm_sb[ct][:, s * XCH:(s + 1) * XCH],
                        start=(ct == 0), stop=(ct == 1))
                nc.scalar.activation(
                    x_sb[:R, s * XCH:(s + 1) * XCH], px[:R, :XCH],
                    mybir.ActivationFunctionType.Relu, bias=bias_sb[:])

            xcols = x_sb[:].rearrange("p (j r) -> p j r", r=H)

            def alloc_slab():
                slab = dfpool.tile([H, K7 * K7 * G * JS], F16, tag="df",
                                   name="dfslab")
                sv = slab[:].rearrange("p (k g j) -> p k g j",
                                       k=K7 * K7, g=G)
                return slab, sv

            def df_column(js, sv, jl, borrow=False):
                """PE: df_j = x_col^T @ W2a^T; ACT: transpose-convert into
                the fp16 slab laid out [96 r, (k, g, j12)]."""
                j = js * JS + jl
                if borrow:
                    dfp = ps.tile([H, 3 * 512], F32, tag="acc", name="px")
                else:
                    dfp = ps.tile([H, 1024], F32, tag="df", name="dfp")
                xc = xcols[:, j, :]
                nc.tensor.matmul(dfp[:, 0:512], xc, w2_sb[:, 0:512],
                                 start=True, stop=True)
                nc.tensor.matmul(dfp[:, 512:NO], xc, w2_sb[:, 512:NO],
                                 start=True, stop=True)
                iv = dfp[:, :NO].rearrange("p (g k) -> p g k", g=G)
                nc.scalar.activation(
                    sv[:, :, :, jl].transpose([0, 2, 1]), iv,
                    mybir.ActivationFunctionType.Copy)

            # prologue: just enough x to start df(0); everything else is
            # deferred and drained at tap slots so PE's in-order stream
            # never blocks the id-add pipeline. df(0) columns start as soon
            # as the x chunk covering them lands; PSUM tags alternate to
            # double-buffer the PE<->ACT chain.
            cur = alloc_slab()
            x_chunk(0, borrow=True)
            for jl in range(5):
                df_column(0, cur[1], jl, borrow=(jl % 2 == 1))
            x_chunk(1, borrow=True)
            for jl in range(5, 10):
                df_column(0, cur[1], jl, borrow=(jl % 2 == 1))
            x_chunk(2, borrow=True)
            for jl in range(10, JS):
                df_column(0, cur[1], jl, borrow=(jl % 2 == 1))

            work = [("x", 3), ("x", 4)]
            nxt = None

            for js in range(NJS):
                # df for slab js+1 is produced column-by-column interleaved
                # into the tap loop so PE/ACT never serialize
                if js + 1 < NJS:
                    nxt = alloc_slab()
                    work.extend(("df", js + 1, nxt[1], jl)
                                for jl in range(JS))
                else:
                    nxt = None
                if js == 0:
                    work.extend(("x", s) for s in range(5, PIX // XCH))
                _, sv = cur
                for ch in range(CH):
                    last_phase = (js == NJS - 1 and ch == CH - 1)
                    # prefetch the next (js, ch) fpad chunk one phase ahead
                    nidx = js * CH + ch + 1
                    fnxt = (fetch_chunk(nidx // CH, nidx % CH)
                            if nidx < NJS * CH else None)
                    acc = ps.tile([H, CHW * JS], F32, tag="acc", name="acc")

                    def id_adds(k, prod):
                        for s in range(CHW * JS // 512):
                            nc.tensor.matmul(
                                acc[:, s * 512:(s + 1) * 512], i96_sb[:],
                                prod[:, s * 512:(s + 1) * 512],
                                start=(k == 0), stop=(k == K7 * K7 - 1))

                    # PE's id-adds are emitted 2 taps behind the mults so
                    # PE always has queued work across phase boundaries
                    addq = []
                    for k in range(K7 * K7):
                        di, dj = divmod(k, K7)
                        # spread ~10/49 tap mults onto the idle Pool engine;
                        # none in the first taps so PE's chain starts hot,
                        # and none at the end of the final phase (a 3.2us
                        # Pool instr there stretches the drain tail)
                        on_pool = (k % 5 == 4 or k == 47)
                        if last_phase:
                            on_pool = (k % 5 == 2 and k < 45)
                        if on_pool:
                            prod = pprodpool.tile([H, CHW * JS], F16,
                                                  tag="pprod", name="pprod")
                        else:
                            prod = prodpool.tile([H, CHW * JS], F16,
                                                 tag="prod", name="prod")
                        in0 = fcur[:, di, :, dj:dj + JS]
                        in1 = sv[:, k, ch * GH:(ch + 1) * GH, :] \
                            .unsqueeze(2).broadcast_to((H, GH, GC, JS))
                        pv = prod[:].rearrange("p (c j) -> p c j", j=JS)
                        eng = nc.gpsimd if on_pool else nc.vector
                        eng.tensor_tensor(pv, in0, in1,
                                          mybir.AluOpType.mult)
                        addq.append((k, prod))
                        if len(addq) > 2:
                            id_adds(*addq.pop(0))
                        if work and k % 4 == 1:
                            item = work.pop(0)
                            if item[0] == "x":
                                x_chunk(item[1])
                            else:
                                df_column(item[1], item[2], item[3])
                    for item in addq:
                        id_adds(*item)
                    ev = outpool.tile([H, CHW * JS], F32, tag="ev",
                                      name="ev")
                    obase = (js * CH + ch) * CHW * JS
                    if last_phase:
                        # pipeline the tail: evac + DMA per 512-slice so the
                        # drain starts as soon as each PSUM bank stops
                        for s in range(CHW * JS // 512):
                            sl = slice(s * 512, (s + 1) * 512)
                            nc.scalar.activation(
                                ev[:, sl], acc[:, sl],
                                mybir.ActivationFunctionType.Copy)
                            nc.sync.dma_start(
                                out=out_d[:, obase + s * 512:
                                          obase + (s + 1) * 512],
                                in_=ev[:, sl])
                    else:
                        nc.scalar.activation(
                            ev[:], acc[:],
                            mybir.ActivationFunctionType.Copy)
                        nc.sync.dma_start(
                            out=out_d[:, obase:obase + CHW * JS],
                            in_=ev[:])
                    fcur = fnxt
                cur = nxt
    if not nc.is_finalized():
        nc.finalize()
    return nc


def _host_weights(W1, bn_gamma, bn_beta, bn_mean, bn_var, W2):
    inv = bn_gamma / np.sqrt(bn_var + BN_EPS)
    w1t = np.ascontiguousarray((W1 * inv[:, None]).T).astype(np.float16)
    bias = (bn_beta - bn_mean * inv).astype(np.float32).reshape(R, 1)
    w2ta = np.zeros((R + 1, G * K7 * K7), np.float16)
    w2ta[:R] = W2.T.astype(np.float16)
    w2ta[R, CENTER::K7 * K7] = 1.0
    i96 = np.eye(H, dtype=np.float16)
    return w1t, bias, w2ta, i96


def _host_fpad7(fm4):
    """[b, 96 r, (js 8, ch 2, di 7, c 128, j 18)] row-shifted fpad chunks."""
    fpad = np.pad(fm4, ((0, 0), (0, 0), (PAD, PAD), (PAD, PAD)))
    fpt = fpad.transpose(0, 2, 1, 3).astype(np.float16)  # [b, 102, 256, 102]
    out = np.empty((B, H, NJS * CH * FCH), np.float16)
    pos = 0
    for js in range(NJS):
        for ch in range(CH):
            for di in range(K7):
                blk = fpt[:, di:di + H, ch * CHW:(ch + 1) * CHW,
                          js * JS:js * JS + FW]
                out[:, :, pos:pos + CHW * FW] = blk.reshape(B, H, -1)
                pos += CHW * FW
    return out


def kernel(feature_map, guide_map, W1, bn_gamma, bn_beta, bn_mean, bn_var, W2):
    fm4 = np.asarray(feature_map, np.float32).reshape(B, C, H, W)
    fpd7 = _host_fpad7(fm4)
    gm = np.ascontiguousarray(
        np.asarray(guide_map, np.float32).reshape(B, C, H, W)
        .transpose(0, 1, 3, 2)).reshape(B, C, PIX).astype(np.float16)
    w1t, bias, w2ta, i96 = _host_weights(
        np.asarray(W1, np.float32), np.asarray(bn_gamma, np.float32),
        np.asarray(bn_beta, np.float32), np.asarray(bn_mean, np.float32),
        np.asarray(bn_var, np.float32), np.asarray(W2, np.float32))

    if "nc" not in _CACHE:
        _CACHE["nc"] = _build_nc()
    nc = _CACHE["nc"]

    ones = np.ones((1, PIX), np.float16)
    in_maps = [dict(fpd7=fpd7[i], gm=gm[i], w1t=w1t, bias=bias,
                    w2ta=w2ta, i96=i96, ones=ones) for i in range(B)]
    _CACHE["in_maps"] = in_maps
    res = bass_utils.run_bass_kernel_spmd(
        nc, in_maps, core_ids=list(range(B)), trace=TRACE)
    _CACHE["last"] = res
    out = np.stack([r["out"] for r in res.results], axis=0)
    # [b, 96 r, (js 8, ch 2, c 128, j 12)] -> (b, 256, 96, 96)
    out = out.reshape(B, H, NJS, CH, CHW, JS).transpose(0, 3, 4, 1, 2, 5)
    return np.ascontiguousarray(out.reshape(B, C, H, W))



# revision 8
# speedup vs baseline: 1.1534x; 1.1534x over previous
"""Trainium2 Bass kernel for DCKModule (involution / dynamic per-pixel conv).

Math (per image, 1 image per core, 8 cores data-parallel over batch):
  x  = relu(W1p @ guide + bias)                  # (64, 9216)
  df = W2 @ x                                    # (784 = 16 g * 49 tap, 9216)
  out[c,r,j] = sum_k df[g(c),k,r,j] * fpad[c, r+di(k), j+dj(k)] + feature

Mapping (fp16 data, fp32 PSUM accumulation), v2 = 128-partition slot tiling:
- Pixel space (96 r x 96 j) is tiled into 24 slots of (32 rows x 12 cols):
  slot s = (window w = s//3, rowgroup b = s%3) covers rows 32b..32b+31 of
  columns 12w..12w+11.  A CHUNK packs 4 consecutive slots on the 128 SBUF
  partitions (slot q at partitions 32q..32q+31), so every DVE/Pool/PE
  instruction runs 128 partitions wide instead of the baseline's 96
  (1.33x more work per billed row).
- Tap row-shift di is materialized host-side (7 shifted copies of the padded
  feature); the col shift dj is a free-dim offset into an 18-wide halo
  window.  The hot loop has NO partition offsets (hw only allows 32-aligned
  operand bases with restrictive span limits).
- Free-dim order in the hot loop is (j, gc, g) with the group index g
  innermost/stride-1 in ALL operands: this lets the per-column df scatter be
  one contiguous (kk,g) run per rowgroup AND keeps every mult operand
  innermost-stride-1 (DVE fp16 2x perf mode).  The x16 group-channel
  broadcast is a stride-0 gc dim on the df operand (free).
- df is produced per image column j on PE ([96 r, 784] PSUM), evacuated to
  fp16 by ACT, then scattered into chunk-slot slabs [128, (j, kk, g)] with
  small contiguous SBUF->SBUF DMAs (DMA may cross partitions; compute
  engines may not).
- Tap mults: DVE (fp16 2x) for 39 taps, Pool for 10.  Tap accumulation:
  PE identity-matmuls into PSUM fp32 (start tap 0, stop tap 48).
- Residual folded into df via a constant-1 x row and a w2a row adding 1.0
  to every group's center tap.  Output fp16; host casts to fp32 and
  unshuffles the slot layout.
"""

import numpy as np

import concourse.bass as bass
import concourse.mybir as mybir
import concourse.tile as tile
from concourse import bacc, bass_utils

B, C, H, W = 8, 256, 96, 96
K7, PAD, G, GC, R = 7, 3, 16, 16, 64
PIX = H * W               # 9216
BN_EPS = 1e-5
JS = 12                   # slot width (output cols per window)
NW = W // JS              # 8 windows
NK = 6                    # chunks (24 slots / 4)
CH = 2                    # channel halves (128 each)
CHW = C // CH             # 128
GH = G // CH              # 8 groups per half
CENTER = PAD * K7 + PAD   # 24
NO = G * K7 * K7          # 784
FW = JS + K7 - 1          # 18: slot cols incl dj halo
FCH = K7 * FW * GC * GH   # 16128: one (k, ch) fpad chunk per partition
NPH = NK * CH             # 12 phases

F32 = mybir.dt.float32
F16 = mybir.dt.float16
TRACE = False

_CACHE = {}

# taps multiplied on Pool (10 of 49): spread out, none at the very start
POOL_TAPS = frozenset({4, 9, 14, 19, 24, 29, 34, 39, 44, 47})


def _slot(k, q):
    """Chunk k, quarter q -> (window, rowgroup)."""
    s = 4 * k + q
    return s // 3, s % 3


def _build_nc():
    nc = bacc.Bacc(None, target_bir_lowering=False)
    fpd_d = nc.dram_tensor("fpd", [128, NPH * FCH], F16, kind="ExternalInput")
    gm_d = nc.dram_tensor("gm", [C, PIX], F16, kind="ExternalInput")
    w1_d = nc.dram_tensor("w1t", [C, R], F16, kind="ExternalInput")
    bias_d = nc.dram_tensor("bias", [R, 1], F32, kind="ExternalInput")
    w2_d = nc.dram_tensor("w2ta", [R + 1, NO], F16, kind="ExternalInput")
    i128_d = nc.dram_tensor("i128", [128, 128], F16, kind="ExternalInput")
    ones_d = nc.dram_tensor("ones", [1, PIX], F16, kind="ExternalInput")
    out_d = nc.dram_tensor("out", [128, NPH * CHW * JS], F16,
                           kind="ExternalOutput")

    with tile.TileContext(nc) as tc:
        with tc.tile_pool(name="persist", bufs=1) as persist, \
             tc.tile_pool(name="gmpool", bufs=2) as gmpool, \
             tc.tile_pool(name="slabpool", bufs=3) as slabpool, \
             tc.tile_pool(name="stagpool", bufs=3) as stagpool, \
             tc.tile_pool(name="prodpool", bufs=5) as prodpool, \
             tc.tile_pool(name="pprodpool", bufs=3) as pprodpool, \
             tc.tile_pool(name="outpool", bufs=2) as outpool, \
             tc.tile_pool(name="fpool", bufs=2) as fpool, \
             tc.tile_pool(name="ps", bufs=1, space="PSUM") as ps:

            w1_sb = persist.tile([128, 2 * R], F16, tag="w1", name="w1sb")
            bias_sb = persist.tile([R, 1], F32, tag="bias", name="biassb")
            w2_sb = persist.tile([R + 1, NO], F16, tag="w2", name="w2sb")
            i128_sb = persist.tile([128, 128], F16, tag="i128", name="i128sb")
            x_sb = persist.tile([R + 1, PIX], F16, tag="x", name="xsb")

            nc.sync.dma_start(out=w1_sb[:, 0:R], in_=w1_d[0:128, :])
            nc.sync.dma_start(out=w1_sb[:, R:2 * R], in_=w1_d[128:256, :])
            nc.sync.dma_start(out=bias_sb[:], in_=bias_d[:])
            nc.sync.dma_start(out=w2_sb[:], in_=w2_d[:])
            nc.sync.dma_start(out=i128_sb[:], in_=i128_d[:])
            # constant-1 row of x folds the +feature residual into df
            nc.sync.dma_start(out=x_sb[R:R + 1, :], in_=ones_d[:])

            def fetch_fp(k, ch):
                fch = fpool.tile([128, FCH], F16, tag="fch", name="fch")
                base = (k * CH + ch) * FCH
                nc.sync.dma_start(out=fch[:], in_=fpd_d[:, base:base + FCH])
                return fch[:].rearrange("p (di jj gc g) -> p di jj gc g",
                                        di=K7, jj=FW, gc=GC)

            fcur = fetch_fp(0, 0)

            # ---- x = relu(W1p @ guide + bias), fp16, pixel-major (j*96+r)
            XCH = 512

            def fetch_gm(s):
                g0 = gmpool.tile([128, XCH], F16, tag="gma", name="gma")
                g1 = gmpool.tile([128, XCH], F16, tag="gmb", name="gmb")
                lo = s * XCH
                nc.sync.dma_start(out=g0[:], in_=gm_d[0:128, lo:lo + XCH])
                nc.sync.dma_start(out=g1[:], in_=gm_d[128:256, lo:lo + XCH])
                return g0, g1

            def x_chunk(s, gtiles):
                px = ps.tile([R, XCH], F32, tag="xps", name="xps")
                for ct in range(2):
                    nc.tensor.matmul(
                        px[:], w1_sb[:, ct * R:(ct + 1) * R], gtiles[ct][:],
                        start=(ct == 0), stop=(ct == 1))
                nc.scalar.activation(
                    x_sb[:R, s * XCH:(s + 1) * XCH], px[:],
                    mybir.ActivationFunctionType.Relu, bias=bias_sb[:])

            def alloc_slab():
                # [128, (j 12, kk 49, g 16)]
                slab = slabpool.tile([128, JS * NO], F16, tag="df",
                                     name="dfslab")
                return slab

            # df production for image column j + DMA scatter into slabs.
            # col j (window w=j//12) scatters rowgroup b to slot 3w+b =
            # chunk (3w+b)//4, partitions 32*((3w+b)%4), one contiguous
            # (kk,g) run of 784 fp16 values per rowgroup.
            def df_col(j, slabs, ab):
                dfp = ps.tile([H, 1024], F32, tag=f"dfps{ab}", name="dfps")
                xc = x_sb[:, j * H:(j + 1) * H]
                nc.tensor.matmul(dfp[:, 0:512], xc, w2_sb[:, 0:512],
                                 start=True, stop=True)
                nc.tensor.matmul(dfp[:, 512:NO], xc, w2_sb[:, 512:NO],
                                 start=True, stop=True)
                stag = stagpool.tile([H, NO], F16, tag="stag", name="stag")
                nc.scalar.activation(stag[:], dfp[:, :NO],
                                     mybir.ActivationFunctionType.Copy)
                w, jl = j // JS, j % JS
                for b in range(3):
                    s = 3 * w + b
                    k, q = s // 4, s % 4
                    if k not in slabs:
                        slabs[k] = alloc_slab()
                    slab = slabs[k]
                    nc.sync.dma_start(
                        out=slab[32 * q:32 * q + 32,
                                 jl * NO:(jl + 1) * NO],
                        in_=stag[32 * b:32 * b + 32, :])

            # ---- prologue -------------------------------------------------
            slabs = {0: alloc_slab(), 1: alloc_slab()}

            gt = {}
            for s in range(3):
                gt[s] = fetch_gm(s)
            for s in range(3):
                x_chunk(s, gt.pop(s))
            for s in range(3, 5):
                gt[s] = fetch_gm(s)
                x_chunk(s, gt.pop(s))
            # chunk 0 needs cols 0..23 (w0 full + w1 rowgroup 0)
            for j in range(24):
                df_col(j, slabs, j % 2)

            # df col j needs x chunk (j*96+95)//512; keep x just ahead of df
            # so cols arrive ~1.5 phases before their consuming chunk.
            work = []
            nx = 5
            for j in range(24, PIX // H):
                while nx <= (j * H + H - 1) // XCH:
                    work.append(("gm", nx))
                    work.append(("x", nx))
                    nx += 1
                work.append(("df", j))
            while nx < PIX // XCH:
                work.append(("gm", nx))
                work.append(("x", nx))
                nx += 1
            nitem = [0]

            def drain_one(slabs):
                if not work:
                    return
                item = work.pop(0)
                nitem[0] += 1
                if item[0] == "gm":
                    gt[item[1]] = fetch_gm(item[1])
                elif item[0] == "x":
                    x_chunk(item[1], gt.pop(item[1]))
                else:
                    df_col(item[1], slabs, nitem[0] % 2)

            # ---- main loop ------------------------------------------------
            for k in range(NK):
                slab = slabs[k]
                sv = slab[:].rearrange("p (j kk g) -> p j kk g",
                                       j=JS, kk=K7 * K7)
                for ch in range(CH):
                    ph = k * CH + ch
                    last_phase = (ph == NPH - 1)
                    nidx = ph + 1
                    fnxt = (fetch_fp(nidx // CH, nidx % CH)
                            if nidx < NPH else None)
                    acc = ps.tile([128, CHW * JS], F32, tag="acc", name="acc")

                    def id_adds(kk, prod):
                        for s in range(CHW * JS // 512):
                            nc.tensor.matmul(
                                acc[:, s * 512:(s + 1) * 512], i128_sb[:],
                                prod[:, s * 512:(s + 1) * 512],
                                start=(kk == 0), stop=(kk == K7 * K7 - 1))

                    addq = []
                    for kk in range(K7 * K7):
                        di, dj = divmod(kk, K7)
                        on_pool = kk in POOL_TAPS
                        if last_phase:
                            on_pool = (kk % 5 == 2 and kk < 45)
                        if on_pool:
                            prod = pprodpool.tile([128, CHW * JS], F16,
                                                  tag="pprod", name="pprod")
                        else:
                            prod = prodpool.tile([128, CHW * JS], F16,
                                                 tag="prod", name="prod")
                        # free order (j, gc, g), g innermost stride 1
                        in0 = fcur[:, di, dj:dj + JS, :, :]
                        in1 = sv[:, :, kk, ch * GH:(ch + 1) * GH] \
                            .unsqueeze(2).broadcast_to((128, JS, GC, GH))
                        pv = prod[:].rearrange("p (j gc g) -> p j gc g",
                                               j=JS, gc=GC)
                        eng = nc.gpsimd if on_pool else nc.vector
                        eng.tensor_tensor(pv, in0, in1,
                                          mybir.AluOpType.mult)
                        addq.append((kk, prod))
                        if len(addq) > 2:
                            id_adds(*addq.pop(0))
                        if kk % 4 == 1:
                            drain_one(slabs)
                    for item in addq:
                        id_adds(*item)
                    ev = outpool.tile([128, CHW * JS], F16, tag="ev",
                                      name="ev")
                    obase = ph * CHW * JS
                    if last_phase:
                        for s in range(CHW * JS // 512):
                            sl = slice(s * 512, (s + 1) * 512)
                            nc.scalar.activation(
                                ev[:, sl], acc[:, sl],
                                mybir.ActivationFunctionType.Copy)
                            nc.sync.dma_start(
                                out=out_d[:, obase + s * 512:
                                          obase + (s + 1) * 512],
                                in_=ev[:, sl])
                    else:
                        nc.scalar.activation(
                            ev[:], acc[:],
                            mybir.ActivationFunctionType.Copy)
                        nc.sync.dma_start(
                            out=out_d[:, obase:obase + CHW * JS],
                            in_=ev[:])
                    fcur = fnxt
                del slabs[k]
    if not nc.is_finalized():
        nc.finalize()
    return nc


def _host_weights(W1, bn_gamma, bn_beta, bn_mean, bn_var, W2):
    inv = bn_gamma / np.sqrt(bn_var + BN_EPS)
    w1t = np.ascontiguousarray((W1 * inv[:, None]).T).astype(np.float16)
    bias = (bn_beta - bn_mean * inv).astype(np.float32).reshape(R, 1)
    # w2a columns ordered (kk-tap major, group minor): col = kk*16 + g
    w2ta = np.zeros((R + 1, NO), np.float16)
    w2 = W2.reshape(G, K7 * K7, R).transpose(1, 0, 2)  # [kk, g, R]
    w2ta[:R] = w2.reshape(NO, R).T.astype(np.float16)
    w2ta[R, CENTER * G:(CENTER + 1) * G] = 1.0
    i128 = np.eye(128, dtype=np.float16)
    return w1t, bias, w2ta, i128


def _host_fpd(fm4):
    """[b, 128, (phase 12, di 7, jj 18, gc 16, g 8)] slot-tiled fpad."""
    fpad = np.pad(fm4, ((0, 0), (0, 0), (PAD, PAD), (PAD, PAD))) \
        .astype(np.float16)  # [b, 256, 102, 102]
    # channels as [ch 2, g 8, gc 16]
    fpg = fpad.reshape(B, CH, GH, GC, H + 2 * PAD, W + 2 * PAD)
    out = np.empty((B, 128, NPH * FCH), np.float16)
    for k in range(NK):
        for q in range(4):
            w, b = _slot(k, q)
            for ch in range(CH):
                base = (k * CH + ch) * FCH
                for di in range(K7):
                    # rows 32b+di .. +32, cols 12w .. +18
                    blk = fpg[:, ch, :, :, 32 * b + di:32 * b + di + 32,
                              12 * w:12 * w + FW]
                    # [b, g 8, gc 16, 32 r, 18 jj] -> [b, 32, jj, gc, g]
                    blk = blk.transpose(0, 3, 4, 2, 1)
                    lo = base + di * FW * GC * GH
                    out[:, 32 * q:32 * q + 32, lo:lo + FW * GC * GH] = \
                        blk.reshape(B, 32, -1)
    return out


def kernel(feature_map, guide_map, W1, bn_gamma, bn_beta, bn_mean, bn_var, W2):
    fm4 = np.asarray(feature_map, np.float32).reshape(B, C, H, W)
    fpd = _host_fpd(fm4)
    gm = np.ascontiguousarray(
        np.asarray(guide_map, np.float32).reshape(B, C, H, W)
        .transpose(0, 1, 3, 2)).reshape(B, C, PIX).astype(np.float16)
    w1t, bias, w2ta, i128 = _host_weights(
        np.asarray(W1, np.float32), np.asarray(bn_gamma, np.float32),
        np.asarray(bn_beta, np.float32), np.asarray(bn_mean, np.float32),
        np.asarray(bn_var, np.float32), np.asarray(W2, np.float32))

    if "nc" not in _CACHE:
        _CACHE["nc"] = _build_nc()
    nc = _CACHE["nc"]

    ones = np.ones((1, PIX), np.float16)
    in_maps = [dict(fpd=fpd[i], gm=gm[i], w1t=w1t, bias=bias,
                    w2ta=w2ta, i128=i128, ones=ones) for i in range(B)]
    _CACHE["in_maps"] = in_maps
    res = bass_utils.run_bass_kernel_spmd(
        nc, in_maps, core_ids=list(range(B)), trace=TRACE)
    _CACHE["last"] = res
    raw = np.stack([r["out"] for r in res.results], axis=0)
    # [b, 128, (phase, j 12, gc 16, g 8)] -> (b, 256, 96, 96)
    raw = raw.reshape(B, 128, NPH, JS, GC, GH).astype(np.float32)
    out = np.empty((B, C, H, W), np.float32)
    for k in range(NK):
        for q in range(4):
            w, b = _slot(k, q)
            for ch in range(CH):
                ph = k * CH + ch
                blk = raw[:, 32 * q:32 * q + 32, ph]  # [b, 32 r, 12 j, gc, g]
                # channel c = 128*ch + g*16 + gc
                blk = blk.transpose(0, 4, 3, 1, 2)    # [b, g, gc, r, j]
                out[:, ch * CHW:(ch + 1) * CHW,
                    32 * b:32 * b + 32,
                    12 * w:12 * w + JS] = blk.reshape(B, CHW, 32, JS)
    return out
